# revision 11
# baseline (speedup 1.0000x reference)
"""GCN layer v2 (h = xW -> sym-normalized scatter-add over edges -> log_softmax)
on 8 Trainium2 NeuronCores.

Sharding: nodes (rows of x / output) sharded across 8 cores; edges partitioned
by destination core; W replicated. Each core computes g = D^-1/2 (x W) for its
node slice (bf16), an AllGather (Shared output) replicates the packed g table
(2 nodes per 256B row) to every core's HBM, then each core dma_gathers g[src]
rows for its edges and scatter-adds them into destination rows with one-hot
mask matmuls on the tensor engine (edges sorted by destination block; source
indices sorted within groups). Self-loop contributions are added directly from
the local g tile (no edges spent on them).
"""
import sys

sys.path.insert(0, "/opt/trn_rl_repo")

import numpy as np
import ml_dtypes

BF16 = ml_dtypes.bfloat16

N = 100000          # nodes
F = 512             # in features
C = 40              # classes
NCORES = 8
NPC = N // NCORES   # nodes per core = 12500
PB = 98             # 128-row blocks per core (12544 padded)
NPAD = PB * 128     # 12544
PBH = PB // 2       # 49 row-pairs per partition
ROWS_PC = 128 * PBH          # 6272 table rows per core (2 nodes/row)
RANGE_ROWS = 4 * ROWS_PC     # 25088 rows per int16 index range (2 ranges)
NRNG = 2
DG = 128            # table row = 128 bf16 = 256B; two nodes (slots 0/1)
G_CH = 64           # chunks (of 128 edges) per dma_gather call (NI=8192)
G_BUFS = 2


def _host_prep(x, edge_index, W, b, g_ch=G_CH):
    x = np.asarray(x, dtype=np.float32)
    W = np.asarray(W, dtype=np.float32)
    b = np.asarray(b, dtype=np.float32)
    src = np.asarray(edge_index[0], dtype=np.int64)
    dst = np.asarray(edge_index[1], dtype=np.int64)

    # degree includes self-loops (+1)
    deg = (np.bincount(dst, minlength=N) + 1).astype(np.int32)

    core = dst // NPC
    dl = dst - core * NPC
    blk = dl >> 7
    dloc = (dl & 127).astype(np.float32)

    # table row for src node: core cs, local i -> partition p=i%128, block j
    # row = cs*ROWS_PC + p*PBH + j//2, slot = j%2
    cs = src // NPC
    i_in = src - cs * NPC
    p_ = i_in & 127
    j_ = i_in >> 7
    row = cs * ROWS_PC + p_ * PBH + (j_ >> 1)
    slot = (j_ & 1).astype(np.int64)
    rng_id = row // RANGE_ROWS
    idx16 = (row - rng_id * RANGE_ROWS).astype(np.int16)

    # group key: (dst core, dst block, range, slot)
    key = ((core * PB + blk) * NRNG + rng_id) * 2 + slot
    order = np.argsort(key, kind="stable")
    s_idx16 = idx16[order]
    s_dloc = dloc[order]
    ngroups = NCORES * PB * NRNG * 2
    cnt = np.bincount(key, minlength=ngroups).reshape(NCORES, PB, NRNG, 2)
    bounds = np.zeros(ngroups + 1, dtype=np.int64)
    np.cumsum(cnt.ravel(), out=bounds[1:])

    n_chunks = np.maximum.reduce(
        [((cnt[c] + 127) >> 7) for c in range(NCORES)]
    )  # [PB, NRNG, 2] shared chunk schedule
    T_r = n_chunks.sum(axis=(0, 2))        # chunks per range
    T_r_pad = ((T_r + g_ch - 1) // g_ch) * g_ch
    K_total = int(n_chunks.sum())

    in_maps = []
    for c in range(NCORES):
        # x^T slice as bf16, padded [F, NPAD]
        xT = np.zeros((F, NPAD), dtype=np.float32)
        xT[:, :NPC] = x[c * NPC:(c + 1) * NPC].T
        xT16 = xT.astype(BF16)
        # deg tiled [128, PB]
        degc = np.ones(NPAD, dtype=np.int32)
        degc[:NPC] = deg[c * NPC:(c + 1) * NPC]
        deg_t = degc.reshape(PB, 128).T.copy()
        # per-range idx streams + dloc matrix
        streams = [np.zeros(128 * int(T_r_pad[r]), dtype=np.int16)
                   for r in range(NRNG)]
        dloc_all = np.full((K_total, 128), 255.0, dtype=np.float32)
        posr = [0] * NRNG
        K = 0
        for b_ in range(PB):
            for r in range(NRNG):
                for s in range(2):
                    nch = int(n_chunks[b_, r, s])
                    if nch == 0:
                        continue
                    gi = ((c * PB + b_) * NRNG + r) * 2 + s
                    k0, k1 = bounds[gi], bounds[gi + 1]
                    m = int(k1 - k0)
                    seg_idx = s_idx16[k0:k1]
                    seg_dloc = s_dloc[k0:k1]
                    if m > 1:
                        o_ = np.argsort(seg_idx, kind="stable")
                        seg_idx = seg_idx[o_]
                        seg_dloc = seg_dloc[o_]
                    st = streams[r]
                    off = 128 * posr[r]
                    st[off:off + m] = seg_idx
                    dl_pad = dloc_all[K:K + nch].reshape(-1)
                    dl_pad[:m] = seg_dloc
                    posr[r] += nch
                    K += nch
        assert K == K_total
        # wrap idx per call of g_ch*128: -> [128, 8*T_r_pad]
        idx_arrs = {}
        for r in range(NRNG):
            if T_r_pad[r] == 0:
                continue
            lin = streams[r].reshape(-1, g_ch * 128)
            wr = lin.reshape(lin.shape[0], -1, 16).transpose(0, 2, 1)
            w16 = np.concatenate(list(wr), axis=1)
            idx_arrs[f"idx{r}"] = np.tile(w16, (8, 1)).copy()
        im = {
            "xT": np.ascontiguousarray(xT16),
            "W": W.astype(BF16),
            "bvec": b.reshape(1, C).copy(),
            "deg": deg_t.copy(),
            "dloc": dloc_all.T.copy(),   # [128, K_total] f32
            **idx_arrs,
        }
        in_maps.append(im)
    return in_maps, n_chunks, T_r_pad, K_total


def _build_program(n_chunks, T_r_pad, K_total, ablate=(),
                   g_ch=G_CH, g_bufs=G_BUFS, repeat=1, single_packet=False):
    import concourse.bass as bass
    import concourse.tile as tile
    from concourse import bacc, mybir, library_config
    from contextlib import ExitStack

    f32 = mybir.dt.float32
    bf16 = mybir.dt.bfloat16
    nc = bacc.Bacc("TRN2", target_bir_lowering=False, debug=False,
                   num_devices=NCORES)

    xT_t = nc.dram_tensor("xT", [F, NPAD], bf16, kind="ExternalInput")
    W_t = nc.dram_tensor("W", [F, C], bf16, kind="ExternalInput")
    b_t = nc.dram_tensor("bvec", [1, C], f32, kind="ExternalInput")
    deg_t = nc.dram_tensor("deg", [128, PB], mybir.dt.int32, kind="ExternalInput")
    dloc_t = nc.dram_tensor("dloc", [128, K_total], f32, kind="ExternalInput")
    idx_ts = {}
    for r in range(NRNG):
        if T_r_pad[r] > 0:
            idx_ts[r] = nc.dram_tensor(f"idx{r}", [128, 8 * int(T_r_pad[r])],
                                       mybir.dt.int16, kind="ExternalInput")
    out_t = nc.dram_tensor("out", [128, PB * C], f32, kind="ExternalOutput")

    with tile.TileContext(nc) as tc, ExitStack() as ctx:
        const = ctx.enter_context(tc.tile_pool(name="const", bufs=1))
        psum = ctx.enter_context(tc.tile_pool(name="psum", bufs=7, space="PSUM"))
        dram = ctx.enter_context(tc.tile_pool(name="dram", bufs=1, space="DRAM"))

        nc.gpsimd.load_library(library_config.mlp)

        # ---- constants ----
        W_sb = const.tile([128, 4, C], bf16)
        nc.sync.dma_start(W_sb[:], W_t[:].rearrange("(a p) c -> p a c", p=128))
        deg_sb = const.tile([128, PB], mybir.dt.int32)
        nc.sync.dma_start(deg_sb[:], deg_t[:])
        degf = const.tile([128, PB], f32)
        nc.vector.tensor_copy(degf[:], deg_sb[:])
        recip = const.tile([128, PB], f32)
        nc.vector.reciprocal(recip[:], degf[:])
        dinv = const.tile([128, PB], f32)
        nc.scalar.activation(dinv[:], recip[:], mybir.ActivationFunctionType.Sqrt)
        iota_i = const.tile([128, 128], mybir.dt.int32)
        nc.gpsimd.iota(iota_i[:], [[1, 128]], channel_multiplier=0)
        iota_f = const.tile([128, 128], bf16)
        nc.vector.tensor_copy(iota_f[:], iota_i[:])
        dloc_sb = const.tile([128, K_total], f32)
        nc.sync.dma_start(dloc_sb[:], dloc_t[:])
        # bias broadcast via ones-matmul
        ones1 = const.tile([1, 128], f32)
        nc.gpsimd.memset(ones1[:], 1.0)
        b_row = const.tile([1, C], f32)
        nc.sync.dma_start(b_row[:], b_t[:])
        b_ps = psum.tile([128, C], f32, space="PSUM", tag="ps")
        nc.tensor.matmul(out=b_ps[:], lhsT=ones1[:], rhs=b_row[:],
                         start=True, stop=True)
        b_bc = const.tile([128, C], f32)
        nc.vector.tensor_copy(b_bc[:], b_ps[:])

        idx_sbs = {}
        for r, t in idx_ts.items():
            tl_ = const.tile([128, 8 * int(T_r_pad[r])], mybir.dt.int16,
                             tag=f"idxfull{r}")
            nc.sync.dma_start(tl_[:], t[:])
            idx_sbs[r] = tl_

        # ---- phase 1: g = dinv * (x @ W), packed 2 nodes per 256B row ----
        # g_sb [128, PB, 64]: block j at [:, j, 0:40]; row-pair layout matches
        # table row = p*PBH + j//2, slot j%2 (contiguous reinterpretation).
        g_sb = const.tile([128, PB, 64], bf16)
        nc.gpsimd.memset(g_sb[:], 0.0)
        JG = 7  # row blocks per xT stripe group (98 = 14*7)
        with tc.tile_pool(name="xt", bufs=8) as xtp:
            for jg in range(PB // JG):
                xts = []
                for kb in range(4):
                    t = xtp.tile([128, JG * 128], bf16, tag="xt")
                    nc.sync.dma_start(
                        t[:], xT_t[kb * 128:(kb + 1) * 128,
                                   jg * JG * 128:(jg + 1) * JG * 128])
                    xts.append(t)
                for jl in range(JG):
                    j = jg * JG + jl
                    ps = psum.tile([128, C], f32, space="PSUM", tag="ps")
                    for kb in range(4):
                        nc.tensor.matmul(
                            out=ps[:],
                            lhsT=xts[kb][:, jl * 128:(jl + 1) * 128],
                            rhs=W_sb[:, kb, :],
                            start=(kb == 0), stop=(kb == 3))
                    nc.vector.tensor_scalar(
                        out=g_sb[:, j, :C], in0=ps[:],
                        scalar1=dinv[:, j:j + 1], scalar2=None,
                        op0=mybir.AluOpType.mult)
        ag_in = dram.tile([128, PB * 64], bf16)
        nc.sync.dma_start(ag_in[:], g_sb[:].rearrange("p a b -> p (a b)"))

        ag_out = dram.tile([NCORES * 128, PB * 64], bf16, addr_space="Shared")
        nc.gpsimd.collective_compute(
            "AllGather", mybir.AluOpType.bypass,
            replica_groups=[list(range(NCORES))],
            ins=[ag_in.opt()], outs=[ag_out.opt()])
        g_view = ag_out[:].rearrange("p (a b) -> (p a) b", b=DG)  # [50176, 128]

        # ---- phase 2: gather + mask-matmul scatter ----
        # repeat>1 is a benchmark-only mode: the identical phase-2 body runs
        # `repeat` times via a hardware loop so device time can be measured
        # as a slope against launch overhead.
        s_sb = const.tile([128, PB, C], f32)
        with tc.tile_pool(name="gath", bufs=g_bufs) as gp, \
             tc.tile_pool(name="mask", bufs=8) as mp, ExitStack() as loop_ctx:
            if repeat > 1:
                loop_ctx.enter_context(tc.For_i(0, repeat))
            posr = [0] * NRNG
            cur = [None] * NRNG
            dummy_g = None
            if "nogather" in ablate:
                dummy_g = const.tile([128, g_ch, DG], bf16)
                nc.gpsimd.memset(dummy_g[:], 0.0)
            K = 0
            for b_ in range(PB):
                ps = psum.tile([128, C], f32, space="PSUM", tag="ps")
                tot = int(n_chunks[b_].sum())
                done = 0
                for r in range(NRNG):
                    for s in range(2):
                        nch = int(n_chunks[b_, r, s])
                        for k in range(nch):
                            pr = posr[r]
                            if "nogather" in ablate:
                                pass
                            elif pr % g_ch == 0:
                                t_call = pr // g_ch
                                it = idx_sbs[r][:, t_call * 8 * g_ch:
                                                (t_call + 1) * 8 * g_ch]
                                gt = gp.tile([128, g_ch, DG], bf16, tag=f"g{r}")
                                nc.gpsimd.dma_gather(
                                    gt[:],
                                    g_view[r * RANGE_ROWS:(r + 1) * RANGE_ROWS, :],
                                    it, g_ch * 128, g_ch * 128, DG,
                                    single_packet=single_packet)
                                cur[r] = gt
                            if "nomask" in ablate:
                                mask = iota_f
                            else:
                                mask = mp.tile([128, 128], bf16, tag="mask")
                                nc.vector.tensor_scalar(
                                    out=mask[:], in0=iota_f[:],
                                    scalar1=dloc_sb[:, K:K + 1], scalar2=None,
                                    op0=mybir.AluOpType.is_equal)
                            rhs_src = (dummy_g if "nogather" in ablate
                                       else cur[r])[:, pr % g_ch,
                                                    s * 64:s * 64 + C]
                            if "nomm" not in ablate:
                                nc.tensor.matmul(
                                    out=ps[:], lhsT=mask[:],
                                    rhs=rhs_src,
                                    start=(done == 0), stop=(done == tot - 1))
                            posr[r] += 1
                            K += 1
                            done += 1
                if "nomm" in ablate:
                    nc.vector.tensor_copy(s_sb[:, b_, :], b_bc[:])
                else:
                    nc.vector.tensor_copy(s_sb[:, b_, :], ps[:])

        # ---- final: out = log_softmax(dinv*(s + g_self) + b) ----
        tmp = const.tile([128, PB, C], f32)
        g98 = g_sb[:]  # [128, PB, 64]; block j's features at [:, j, :C]
        nc.vector.tensor_tensor(out=tmp[:], in0=s_sb[:], in1=g98[:, :, :C],
                                op=mybir.AluOpType.add)
        dinv_bc = dinv[:].unsqueeze(2).to_broadcast([128, PB, C])
        nc.vector.tensor_tensor(out=tmp[:], in0=tmp[:], in1=dinv_bc,
                                op=mybir.AluOpType.mult)
        b_bc_ap = b_bc[:].unsqueeze(1).to_broadcast([128, PB, C])
        nc.vector.tensor_tensor(out=tmp[:], in0=tmp[:], in1=b_bc_ap,
                                op=mybir.AluOpType.add)
        rmax = const.tile([128, PB], f32)
        nc.vector.tensor_reduce(out=rmax[:], in_=tmp[:],
                                axis=mybir.AxisListType.X, op=mybir.AluOpType.max)
        rmax_bc = rmax[:].unsqueeze(2).to_broadcast([128, PB, C])
        nc.vector.tensor_tensor(out=tmp[:], in0=tmp[:], in1=rmax_bc,
                                op=mybir.AluOpType.subtract)
        esb = const.tile([128, PB, C], f32)
        nc.scalar.activation(esb[:], tmp[:], mybir.ActivationFunctionType.Exp)
        ssum = const.tile([128, PB], f32)
        nc.vector.tensor_reduce(out=ssum[:], in_=esb[:],
                                axis=mybir.AxisListType.X, op=mybir.AluOpType.add)
        lse = const.tile([128, PB], f32)
        nc.scalar.activation(lse[:], ssum[:], mybir.ActivationFunctionType.Ln)
        lse_bc = lse[:].unsqueeze(2).to_broadcast([128, PB, C])
        nc.vector.tensor_tensor(out=tmp[:], in0=tmp[:], in1=lse_bc,
                                op=mybir.AluOpType.subtract)
        nc.sync.dma_start(out_t[:], tmp[:].rearrange("p a b -> p (a b)"))

    nc.compile()
    return nc


_CACHE = {}


def _get_program(n_chunks, T_r_pad, K_total, g_ch=G_CH, g_bufs=G_BUFS):
    key = (n_chunks.tobytes(), tuple(int(t) for t in T_r_pad), g_ch, g_bufs)
    if key not in _CACHE:
        _CACHE[key] = _build_program(n_chunks, T_r_pad, K_total,
                                     g_ch=g_ch, g_bufs=g_bufs)
    return _CACHE[key]


def kernel(x, edge_index, W, b, _trace=False):
    from concourse.bass_utils import run_bass_kernel_spmd

    in_maps, n_chunks, T_r_pad, K_total = _host_prep(x, edge_index, W, b)
    nc = _get_program(n_chunks, T_r_pad, K_total)
    res = run_bass_kernel_spmd(nc, in_maps, core_ids=list(range(NCORES)),
                               trace=_trace)
    out = np.empty((N, C), dtype=np.float32)
    for c in range(NCORES):
        o = res.results[c]["out"].reshape(128, PB, C)
        out[c * NPC:(c + 1) * NPC] = o.transpose(1, 0, 2).reshape(NPAD, C)[:NPC]
    if _trace:
        return out, res
    return out
